# revision 1
# baseline (speedup 1.0000x reference)
"""Grouped-GEMM (MoE expert FFN) kernel for 8 Trainium2 NeuronCores.

Problem: out[e, m, n] = sum_k x[e, m, k] * w[e, n, k] for m < m_sizes[e],
         zero elsewhere.  E=8, MAX_M=2048, K=2048, N=8192, fp32.

Design: exact-m, w-stationary / xT-moving, bf16, tuned DMA overlap.
----------------------------------------------------------------------
* N-split sharding: every core computes ALL experts against its own
  (N/8)=1024-wide column slice of each expert's weights (perfect load
  balance; each weight element read once fleet-wide).
* The PE's stationary operand is a [128k, 128n] weight tile and the
  MOVING operand is x^T [128k, mb<=512 rows].  The moving free dim is
  the token count, so the kernel processes EXACTLY m_e rows per expert
  (no ceil(m/128) row-tile padding).  m-blocks are sized evenly so
  every block is >=253 rows and LDWEIGHTS stays hidden.
* bf16 operands: 1 cycle/row on the PE (same rate as fp32r) at HALF
  the HBM traffic; output stored bf16 (tolerance 2e-2, bf16 path is
  ~4e-3) and un-transposed on the host.
* Queue discipline (one DMA queue per pool so a trigger waiting on
  pool rotation never blocks an unrelated stream):
    sync   = weights (w_bufs=20 => 2.5 experts of prefetch; covers the
             small trailing experts),
    scalar = x blocks (two half-K tiles per block),
    gpsimd = output + first expert's odd w tiles (parallelises the
             critical first-expert weight load; x must NOT ride this
             queue -- it is slow (~134GB/s) and starts late),
    vector = PSUM->SBUF bf16 casts only.
* Output is staged per m-block ([128, 8nn*mb] tile, one strided DMA)
  -- 19 output DMAs instead of 152, shrinking semaphore churn.
"""
import sys
import types

import ml_dtypes
import numpy as np

import concourse.bass as bass
import concourse.tile as tile
from concourse import bacc, mybir
from concourse.bass_utils import run_bass_kernel_spmd

P = 128          # partition dim / k-tile
N_CORES = 8
MB = 512         # max moving rows per matmul (one PSUM bank of fp32)

LAST_RESULT = None   # BassKernelResults of the most recent run (for tests)


def _install_profile_shim():
    """The agent image's antenv stub lacks axon_hooks; provide it so
    BASS_TRACE=1 profiling works instead of crashing."""
    if "antenv.axon_hooks" in sys.modules:
        return
    try:
        from trn_agent_boot.trn_boot import _ntff_profile_via_ctypes
        hook = _ntff_profile_via_ctypes("/opt/axon/libaxon_pjrt.so")
        mod = types.ModuleType("antenv.axon_hooks")
        mod.get_axon_ntff_profile_hook = lambda: hook
        sys.modules["antenv.axon_hooks"] = mod
        import antenv
        antenv.axon_hooks = mod
    except Exception:
        pass


def to_bf16(a: np.ndarray) -> np.ndarray:
    return np.asarray(a, dtype=ml_dtypes.bfloat16)


def blocks_of(m):
    """Even m-block sizes: ceil(m/MB) blocks, sizes differing by <=1."""
    nb = (m + MB - 1) // MB
    base = m // nb
    rem = m - base * nb
    return [base + (1 if i < rem else 0) for i in range(nb)]


def build_nc(m_list, K, NC_N, psum_bufs=8, w_bufs=20, x_bufs=6, out_bufs=3):
    """SPMD program for per-segment (expert) valid row counts m_list."""
    KK = K // P
    KH = KK // 2
    NN = NC_N // P
    SM = sum(m_list)

    nc = bacc.Bacc("TRN2", target_bir_lowering=False, debug=False,
                   num_devices=N_CORES)
    n_blocks = sum(len(blocks_of(m)) for m in m_list)
    # x^T, packed per (segment, m-block): [128, KK*mbs] used cols
    xsw = nc.dram_tensor("xsw", [n_blocks * P, KK * MB], mybir.dt.bfloat16,
                         kind="ExternalInput").ap()
    # weights, packed per (segment, nn): row ((seg*NN + nn)*P + p),
    # col kk*P + j  =  w[seg, nn*P + j, kk*P + p]
    wsw = nc.dram_tensor("wsw", [len(m_list) * NN * P, KK * P],
                         mybir.dt.bfloat16, kind="ExternalInput").ap()
    # output, transposed+interleaved: [p, nn*SM + m] = out[m, nn*P + p]
    out = nc.dram_tensor("out", [P, NN * SM], mybir.dt.bfloat16,
                         kind="ExternalOutput").ap()
    out3 = out.rearrange("p (a m) -> p a m", a=NN)

    with tile.TileContext(nc) as tc:
        with tc.tile_pool(name="wp", bufs=w_bufs) as wp, \
             tc.tile_pool(name="xp", bufs=x_bufs) as xp, \
             tc.tile_pool(name="op", bufs=out_bufs) as op, \
             tc.tile_pool(name="pp", bufs=psum_bufs, space="PSUM") as pp, \
             tc.tile_pool(name="wu", bufs=1) as wu:
            # PE warmup: dummy bf16 matmuls spanning the initial DMA wait
            # keep the HAM activity monitor engaged so the PE clock is at
            # 2.4 GHz when the first real tiles land.
            wa_r = wu.tile([P, MB], mybir.dt.bfloat16, tag="war")
            nc.gpsimd.memset(wa_r[:], 0.0)
            wpss = [pp.tile([P, MB], mybir.dt.float32, tag="ps",
                            name="wps") for _ in range(4)]
            for i in range(12):
                nc.tensor.matmul(wpss[i % 4][:], wa_r[:, :P], wa_r[:],
                                 start=True, stop=True)
            blk = 0
            col0 = 0
            for seg, m in enumerate(m_list):
                w_ts = []
                for nn in range(NN):
                    w_t = wp.tile([P, KK * P], mybir.dt.bfloat16, tag="w")
                    # first expert: odd nn tiles ride the (idle) gpsimd
                    # queue so the 4MB expert load halves in latency
                    eng = nc.gpsimd if (seg == 0 and nn % 2 == 1) else nc.sync
                    eng.dma_start(
                        out=w_t[:],
                        in_=wsw[(seg * NN + nn) * P:(seg * NN + nn + 1) * P,
                                :])
                    w_ts.append(w_t)
                for bi, mbs in enumerate(blocks_of(m)):
                    xa = xp.tile([P, KH * MB], mybir.dt.bfloat16, tag="xa")
                    xb = xp.tile([P, KH * MB], mybir.dt.bfloat16, tag="xb")
                    nc.scalar.dma_start(
                        out=xa[:, :KH * mbs],
                        in_=xsw[blk * P:(blk + 1) * P, :KH * mbs])
                    nc.scalar.dma_start(
                        out=xb[:, :KH * mbs],
                        in_=xsw[blk * P:(blk + 1) * P, KH * mbs:KK * mbs])
                    blk += 1
                    # last segment: per-nn output DMAs overlap the final
                    # casts so the kernel tail is one small DMA, not one
                    # big consolidated one
                    tail_seg = seg == len(m_list) - 1
                    o_t = op.tile([P, NN * MB], mybir.dt.bfloat16, tag="o")
                    for nn in range(NN):
                        ps = pp.tile([P, MB], mybir.dt.float32, tag="ps",
                                     name="ps")
                        for kk in range(KK):
                            xt = xa if kk < KH else xb
                            j = kk - KH if kk >= KH else kk
                            nc.tensor.matmul(
                                ps[:, :mbs],
                                w_ts[nn][:, kk * P:(kk + 1) * P],
                                xt[:, j * mbs:(j + 1) * mbs],
                                start=(kk == 0), stop=(kk == KK - 1))
                        nc.vector.tensor_copy(
                            o_t[:, nn * mbs:(nn + 1) * mbs], ps[:, :mbs])
                        if tail_seg:
                            nc.gpsimd.dma_start(
                                out=out3[:, nn, col0:col0 + mbs],
                                in_=o_t[:, nn * mbs:(nn + 1) * mbs])
                    if not tail_seg:
                        nc.gpsimd.dma_start(
                            out=out3[:, :, col0:col0 + mbs],
                            in_=o_t[:, :NN * mbs])
                    col0 += mbs
    nc.compile()
    return nc


_NC_CACHE = {}


def get_nc(m_list, K, NC_N, **kw):
    key = (tuple(m_list), K, NC_N, tuple(sorted(kw.items())))
    if key not in _NC_CACHE:
        _NC_CACHE[key] = build_nc(m_list, K, NC_N, **kw)
    return _NC_CACHE[key]


def pack_x(x_padded, order, m_all, K):
    """x^T packed per (expert, m-block): [128, KK*mbs] rows, padded to
    the fixed [P, KK*MB] dram row width."""
    KK = K // P
    parts = []
    for e in order:
        m = m_all[e]
        mb0 = 0
        for mbs in blocks_of(m):
            b = x_padded[e, mb0:mb0 + mbs, :].T        # [K, mbs]
            b = b.reshape(KK, P, mbs).transpose(1, 0, 2)  # [P, KK, mbs]
            row = np.zeros((P, KK * MB), dtype=np.float32)
            row[:, :KK * mbs] = b.reshape(P, KK * mbs)
            parts.append(row)
            mb0 += mbs
    return to_bf16(np.concatenate(parts, axis=0))


def pack_w(stacked_weights, order, c, NC_N, K):
    """Weights per (expert, nn): [128, KK*P] with
    [p, kk*P+j] = w[e, c*NC_N + nn*P + j, kk*P + p]."""
    KK = K // P
    NN = NC_N // P
    parts = []
    for e in order:
        blk = stacked_weights[e, c * NC_N:(c + 1) * NC_N, :]  # [NC_N, K]
        a = blk.reshape(NN, P, KK, P).transpose(0, 3, 2, 1)   # [nn, p, kk, j]
        parts.append(np.ascontiguousarray(a).reshape(NN * P, KK * P))
    return to_bf16(np.concatenate(parts, axis=0))


def kernel(x_padded, stacked_weights, m_sizes):
    global LAST_RESULT
    x_padded = np.ascontiguousarray(np.asarray(x_padded, dtype=np.float32))
    stacked_weights = np.ascontiguousarray(
        np.asarray(stacked_weights, dtype=np.float32))
    E, MAX_M, K = x_padded.shape
    N = stacked_weights.shape[1]
    NC_N = N // N_CORES
    NN = NC_N // P
    m_all = [int(min(max(int(mm), 0), MAX_M))
             for mm in np.asarray(m_sizes).astype(np.int64)]

    out_full = np.zeros((E, MAX_M, N), dtype=np.float32)
    order = [e for e in range(E) if m_all[e] > 0]
    if not order:
        return out_full
    # descending size: the big first expert hides its own weight load,
    # and the small trailing experts ride 3-expert weight prefetch
    order.sort(key=lambda e: -m_all[e])
    m_list = [m_all[e] for e in order]
    SM = sum(m_list)

    _install_profile_shim()
    nc = get_nc(m_list, K, NC_N)

    xsw = pack_x(x_padded, order, m_all, K)
    in_maps = [{"xsw": xsw,
                "wsw": pack_w(stacked_weights, order, c, NC_N, K)}
               for c in range(N_CORES)]

    res = run_bass_kernel_spmd(nc, in_maps, list(range(N_CORES)))
    LAST_RESULT = res

    for c in range(N_CORES):
        o = np.asarray(res.results[c]["out"]).astype(np.float32)  # [P, NN*SM]
        outT = o.reshape(P, NN, SM).transpose(1, 0, 2).reshape(NC_N, SM)
        col = 0
        for i, e in enumerate(order):
            m = m_list[i]
            out_full[e, :m, c * NC_N:(c + 1) * NC_N] = outT[:, col:col + m].T
            col += m
    return out_full

